# revision 5
# baseline (speedup 1.0000x reference)
"""Trainium2 Bass kernel for geodesic convolution (gnn_message_passing).

Computation (per vertex v, with m = (i,j) flattened, NR*NT = 40 slots):
  x[v,m,c]    = sum_t bary_w[v,m,t] * signal[bary_idx[v,m,t], c]
  conv[v,k,d] = sum_{m,c} x[v,m,c] * K[i(m),(j(m)+k)%NT,c,d]
  out[v,:]    = relu(conv[v, argmax_k ||conv[v,k,:]||, :])

Strategy: shard V across 8 cores. The signal stays SBUF-resident in
channel-major transposed form [128 partitions = (h, c), 25024 rows] f32,
split into two row-halves h (int16 gather index limit). The gather runs on
GPSIMD (ap_gather): each Q7 core gathers along its partitions' free dim, so
a position's 64 channels land on 64 partitions at the same free offset.
Positions whose row lies in the other half gather a dummy row and are
masked by zero weights (2-row weight tensor broadcast to 128 partitions by
a tiny PE matmul per 480-column chunk). After the weighted tap-sum, the two
half contributions are merged (SBUF->SBUF DMA partition shift + DVE add)
and re-split by ij-parity into the 128-row lhsT chunks of a 20-chunk f32
matmul accumulation against the pre-rotated kernel matrix. Everything runs
inside tc.For_i hardware loops; tiles are processed in 4 quarters with two
gather buffers so the GPSIMD gather of quarter q+1 overlaps the
PE/DVE/DMA compute of quarter q.
"""

import numpy as np

# Problem constants (hardcoded; kernel.py must be self-contained).
V, NR, NT, CIN, COUT = 50000, 5, 8, 64, 64
NCORES = 8
VPC = V // NCORES            # 6250 vertices per core
TPT = 128                    # vertices per tile
NTILES = -(-VPC // TPT)      # 49
VPAD = NTILES * TPT          # 6272
M = NR * NT                  # 40 (i,j) slots
NPOS = M * TPT * 3           # 15360 gather positions per tile
HALF = 25024                 # rows per signal half ((h, c) partition split)
NQ = 4                       # quarters per tile
MQ = M // NQ                 # 10 m-slots per quarter
QPOS = NPOS // NQ            # 3840 positions per quarter
CCH = 480                    # weight-broadcast chunk columns
NCH = QPOS // CCH            # 8 chunks per quarter
KD = NT * COUT               # 512 output cols (k,d)
NPAIR = M // 2               # 20 matmul chunks (ij-parity pairs)

_CACHE = {}


def build_program(repeat=1):
    """Build the Bacc program for one SPMD core. Returns compiled nc.

    repeat is a For_i loop bound around the whole tile loop; static program
    size is identical for any repeat, so the wall-clock slope between
    repeat values measures pure per-pass hardware execution time.
    """
    import concourse.bass as bass
    import concourse.mybir as mybir
    import concourse.tile as tile
    from concourse import bacc
    from concourse.bass import ts

    f32 = mybir.dt.float32
    i16 = mybir.dt.int16

    nc = bacc.Bacc(
        "TRN2",
        target_bir_lowering=False,
        debug=False,
        enable_asserts=False,
        num_devices=NCORES,
    )
    from concourse.bass import ds

    sig_d = nc.dram_tensor("sig2", [128, HALF], f32, kind="ExternalInput")
    idx_d = nc.dram_tensor("idx16", [NTILES * TPT, NPOS // 16], i16,
                           kind="ExternalInput")
    wv_d = nc.dram_tensor("wv2", [NTILES * 2, NPOS], f32,
                          kind="ExternalInput")
    sel_d = nc.dram_tensor("sel", [2, 128], f32, kind="ExternalInput")
    wm_d = nc.dram_tensor("wm2", [128, NPAIR * KD], f32, kind="ExternalInput")
    out_d = nc.dram_tensor("out", [VPAD, COUT], f32, kind="ExternalOutput")

    with tile.TileContext(nc) as tc:
        with (
            tc.tile_pool(name="const", bufs=1) as cpool,
            tc.tile_pool(name="io", bufs=1) as iopool,
            tc.tile_pool(name="g", bufs=1) as gpool,
            tc.tile_pool(name="lt", bufs=1) as ltpool,
            tc.tile_pool(name="epi", bufs=1) as epool,
            tc.tile_pool(name="psA", bufs=2, space="PSUM") as psA,
            tc.tile_pool(name="psB", bufs=2, space="PSUM") as psB,
        ):
            sig_t = cpool.tile([128, HALF], f32)
            nc.sync.dma_start(out=sig_t[:], in_=sig_d.ap())
            wm_t = cpool.tile([128, NPAIR, KD], f32)
            nc.sync.dma_start(
                out=wm_t[:], in_=wm_d.ap().rearrange("p (m n) -> p m n",
                                                     m=NPAIR))

            sel_t = cpool.tile([2, 128], f32)
            nc.sync.dma_start(out=sel_t[:], in_=sel_d.ap())
            i_a = iopool.tile([128, NPOS // 16], i16, tag="ia")
            i_b = iopool.tile([128, NPOS // 16], i16, tag="ib")
            wv_h0 = iopool.tile([2, QPOS // 2], f32, tag="wva")
            wv_h1 = iopool.tile([2, QPOS // 2], f32, tag="wvb")
            wv_h = [wv_h0, wv_h1]
            g0 = gpool.tile([128, QPOS], f32, tag="g0")
            g1 = gpool.tile([128, QPOS], f32, tag="g1")
            lt_a = ltpool.tile([128, NPAIR // NQ, 128], f32, tag="la")
            lt_b = ltpool.tile([128, NPAIR // NQ, 128], f32, tag="lb")
            lt_h = ltpool.tile([128, NPAIR // NQ, 128], f32, tag="lh")
            sq_t = epool.tile([128, KD], f32, tag="sq")
            norm_t = epool.tile([128, NT], f32, tag="norm")
            mx_t = epool.tile([128, 1], f32, tag="mx")
            mask_t = epool.tile([128, NT], f32, tag="mask")
            msel_t = epool.tile([128, NT, COUT], f32, tag="msel")
            o_t = epool.tile([128, COUT], f32, tag="o")

            XT2 = QPOS // 3          # 1280: tap-summed cols per quarter
            gbufs = [g0, g1]

            def gather(gq, i_t):
                q = gq % NQ
                g_t = gbufs[gq % 2]
                nc.gpsimd.ap_gather(
                    out_ap=g_t[:].unsqueeze(-1),
                    in_ap=sig_t[:].unsqueeze(-1),
                    idxs_ap=i_t[:, q * (QPOS // 16):(q + 1) * (QPOS // 16)],
                    channels=128, num_elems=HALF, d=1, num_idxs=QPOS,
                )

            def compute(gq, t, conv_p, lt_t):
                q = gq % NQ
                g_t = gbufs[gq % 2]
                # stage the (lo-masked, hi-masked) weight rows, then expand
                # them to the matching partition halves with tiny PE matmuls
                # (sel[h',p] = (p div 64 == h')), multiplying in-place per
                # 480-column PSUM chunk.
                for hh_ in range(2):
                    hs = slice(q * QPOS + hh_ * (QPOS // 2),
                               q * QPOS + (hh_ + 1) * (QPOS // 2))
                    nc.sync.dma_start(out=wv_h[hh_][:],
                                      in_=wv_d.ap()[ds(t * 2, 2), hs])
                for ch in range(NCH):
                    pw = psB.tile([128, CCH], f32, tag="pw")
                    wvb = wv_h[ch // (NCH // 2)]
                    cc = ch % (NCH // 2)
                    nc.tensor.matmul(
                        pw[:], lhsT=sel_t[:],
                        rhs=wvb[:, cc * CCH:(cc + 1) * CCH],
                        start=True, stop=True,
                    )
                    nc.vector.tensor_tensor(
                        out=g_t[:, ch * CCH:(ch + 1) * CCH],
                        in0=g_t[:, ch * CCH:(ch + 1) * CCH],
                        in1=pw[:],
                        op=mybir.AluOpType.mult,
                    )
                # tap-sum into g[:, :XT2] (in-place strided: write n < read 3n)
                nc.vector.tensor_reduce(
                    out=g_t[:, :XT2],
                    in_=g_t[:].rearrange("p (n t) -> p n t", t=3),
                    axis=mybir.AxisListType.X, op=mybir.AluOpType.add,
                )
                # h-merge + o-split: 4 independent SBUF->SBUF DMAs copy the
                # lo/hi half contributions, ij-parity-split, into two
                # [(o,c), k', v] tiles, then one DVE add merges the halves.
                # (Flatter than shift->add->split, and g is released sooner.)
                xlo = g_t[0:64, 0:XT2].rearrange("p (a v) -> p a v", v=128)
                xhi = g_t[64:128, 0:XT2].rearrange("p (a v) -> p a v", v=128)
                nc.sync.dma_start(out=lt_t[0:64, :, :], in_=xlo[:, 0::2, :])
                nc.sync.dma_start(out=lt_t[64:128, :, :], in_=xlo[:, 1::2, :])
                nc.sync.dma_start(out=lt_h[0:64, :, :], in_=xhi[:, 0::2, :])
                nc.sync.dma_start(out=lt_h[64:128, :, :], in_=xhi[:, 1::2, :])
                nc.vector.tensor_tensor(
                    out=lt_t[:], in0=lt_t[:], in1=lt_h[:],
                    op=mybir.AluOpType.add,
                )
                for kp in range(NPAIR // NQ):
                    mp = q * (NPAIR // NQ) + kp
                    nc.tensor.matmul(
                        conv_p[:], lhsT=lt_t[:, kp, :], rhs=wm_t[:, mp, :],
                        start=(mp == 0), stop=(mp == NPAIR - 1),
                    )

            def epilogue(conv_p, t):
                # norms, argmax via is_equal mask, select, relu, store.
                nc.scalar.activation(
                    out=sq_t[:], in_=conv_p[:],
                    func=mybir.ActivationFunctionType.Square,
                )
                nc.vector.tensor_reduce(
                    out=norm_t[:],
                    in_=sq_t[:].rearrange("p (k d) -> p k d", d=COUT),
                    axis=mybir.AxisListType.X, op=mybir.AluOpType.add,
                )
                nc.vector.tensor_reduce(
                    out=mx_t[:], in_=norm_t[:],
                    axis=mybir.AxisListType.X, op=mybir.AluOpType.max,
                )
                nc.vector.tensor_scalar(
                    out=mask_t[:], in0=norm_t[:], scalar1=mx_t[:],
                    scalar2=None, op0=mybir.AluOpType.is_equal,
                )
                nc.vector.tensor_tensor(
                    out=msel_t[:],
                    in0=conv_p[:].rearrange("p (k d) -> p k d", d=COUT),
                    in1=mask_t[:].unsqueeze(-1).to_broadcast(
                        [128, NT, COUT]),
                    op=mybir.AluOpType.mult,
                )
                nc.vector.tensor_reduce(
                    out=o_t[:],
                    in_=msel_t[:].rearrange("p k d -> p d k"),
                    axis=mybir.AxisListType.X, op=mybir.AluOpType.add,
                )
                nc.vector.tensor_scalar_max(o_t[:], o_t[:], 0.0)
                nc.sync.dma_start(out=out_d.ap()[ts(t, TPT), :], in_=o_t[:])

            def tile_pair(te, to):
                # 8 quarters across two tiles; the gather of quarter gq+1 is
                # issued before the compute of quarter gq so GPSIMD never
                # idles, including across the tile boundary.
                nc.sync.dma_start(out=i_a[:], in_=idx_d.ap()[ts(te, TPT), :])
                nc.sync.dma_start(out=i_b[:], in_=idx_d.ap()[ts(to, TPT), :])
                conv_e = psA.tile([128, KD], f32, tag="conv")
                conv_o = psA.tile([128, KD], f32, tag="conv")
                parts = [(i_a, te, conv_e, lt_a), (i_b, to, conv_o, lt_b)]
                gather(0, i_a)
                for gq in range(1, 2 * NQ):
                    gather(gq, parts[gq // NQ][0])
                    i_, t_, c_, l_ = parts[(gq - 1) // NQ]
                    compute(gq - 1, t_, c_, l_)
                    if gq - 1 == NQ - 1:
                        epilogue(conv_e, te)
                compute(2 * NQ - 1, to, conv_o, lt_b)
                epilogue(conv_o, to)

            with tc.For_i(0, repeat) as r:
                with tc.For_i(0, NTILES // 2) as hh:
                    tile_pair(hh * 2, hh * 2 + 1)
                # tail tile (NTILES is odd)
                tl = NTILES - 1
                nc.sync.dma_start(out=i_a[:], in_=idx_d.ap()[ts(tl, TPT), :])
                conv_t = psA.tile([128, KD], f32, tag="conv")
                gather(0, i_a)
                for q in range(NQ):
                    if q + 1 < NQ:
                        gather(q + 1, i_a)
                    compute(q, tl, conv_t, lt_a)
                epilogue(conv_t, tl)

    nc.compile()
    return nc


def _host_prep(signal, bary_w, bary_idx, kernel):
    """Build per-core input maps. All host-side numpy, not timed."""
    sig = np.asarray(signal, np.float32)
    sig2 = np.zeros((128, HALF), np.float32)
    sig2[:64, :HALF] = sig[:HALF].T
    sig2[64:, :V - HALF] = sig[HALF:].T

    jj = np.arange(NT)
    rot = kernel[:, (jj[:, None] + jj[None, :]) % NT, :, :]  # [i,j,k,c,d]
    wm = np.ascontiguousarray(
        rot.transpose(0, 1, 3, 2, 4).reshape(M * CIN, KD), np.float32
    )  # row (m, c)
    wm2 = np.ascontiguousarray(
        wm.reshape(NPAIR, 2, CIN, KD).transpose(1, 2, 0, 3)
        .reshape(128, NPAIR * KD), np.float32
    )  # row (o*64 + c), cols (m_pair, kd)

    idx_full = bary_idx.reshape(V, M, 3).astype(np.int32)
    wv_full = bary_w.reshape(V, M, 3).astype(np.float32)

    in_maps = []
    for c in range(NCORES):
        sl = slice(c * VPC, (c + 1) * VPC)
        idx_c = np.zeros((VPAD, M, 3), np.int32)
        idx_c[:VPC] = idx_full[sl]
        wv_c = np.zeros((VPAD, M, 3), np.float32)
        wv_c[:VPC] = wv_full[sl]

        # position order within a tile: n = m*384 + v*3 + t
        r = (idx_c.reshape(NTILES, TPT, M, 3)
             .transpose(0, 2, 1, 3).reshape(NTILES, NPOS))
        w = (wv_c.reshape(NTILES, TPT, M, 3)
             .transpose(0, 2, 1, 3).reshape(NTILES, NPOS))

        lo = np.where(r < HALF, r, 0).astype(np.int16)
        hi = np.where(r >= HALF, r - HALF, 0).astype(np.int16)
        idx16 = np.zeros((NTILES, 128, NPOS // 16), np.int16)
        # per quarter, wrapped in 16 within each quarter's index range
        for q in range(NQ):
            s = slice(q * QPOS, (q + 1) * QPOS)
            cs = slice(q * (QPOS // 16), (q + 1) * (QPOS // 16))
            wlo = lo[:, s].reshape(NTILES, QPOS // 16, 16).transpose(0, 2, 1)
            whi = hi[:, s].reshape(NTILES, QPOS // 16, 16).transpose(0, 2, 1)
            idx16[:, :64, cs] = np.tile(wlo, (1, 4, 1))
            idx16[:, 64:, cs] = np.tile(whi, (1, 4, 1))

        wv2 = np.zeros((NTILES, 2, NPOS), np.float32)
        wv2[:, 0] = w * (r < HALF)
        wv2[:, 1] = w * (r >= HALF)
        sel = np.zeros((2, 128), np.float32)
        sel[0, :64] = 1.0
        sel[1, 64:] = 1.0

        in_maps.append({
            "sig2": sig2,
            "idx16": idx16.reshape(NTILES * TPT, NPOS // 16),
            "wv2": wv2.reshape(NTILES * 2, NPOS),
            "sel": sel,
            "wm2": wm2,
        })
    return in_maps


def kernel(signal, bary_w, bary_idx, kernel):
    from concourse.bass_utils import run_bass_kernel_spmd

    if "nc" not in _CACHE:
        _CACHE["nc"] = build_program()
    nc = _CACHE["nc"]
    in_maps = _host_prep(signal, bary_w, bary_idx, kernel)
    res = run_bass_kernel_spmd(nc, in_maps, core_ids=list(range(NCORES)))
    out = np.concatenate(
        [res.results[c]["out"][:VPC] for c in range(NCORES)], axis=0
    )
    return out.astype(np.float32)
